# revision 23
# baseline (speedup 1.0000x reference)
"""DSimilarity.gradgrad force-force covariance block on 8 Trainium2 cores.

out[m*3+a, n*3+b] = sum_{i,j} u1[i,a]*u2[j,b]*gg[i,j]*[i1[i]==m]*[i2[j]==n]
with gg[i,j] = (c - c^2 diff^2) * exp(-0.5 c diff^2), diff = d1[i]-d2[j], c=1/l^2.

gg depends only on the scalar difference d1[i]-d2[j], so the 4000x4000 kernel
matrix separates: a 2D Chebyshev expansion of gg on the observed d-range,
truncated by SVD, gives gg ~= sum_k phi_k(d1) psi_k(d2) with rank ~16 at
machine precision (rank 32 used for margin). Folding the scatter matrices in
on the host reduces the whole computation to out = M^T @ W with
M[k, 3m+a] = sum_{i1[i]=m} phi_k(d1_i) u1[i,a]
W[k, 3n+b] = sum_{i2[j]=n} psi_k(d2_j) u2[j,b].

The 1536x1536 (padded) output is tiled into 24 pieces: 12 column chunks of
128 (transposed: out columns on PSUM partitions) x 2 row halves of 768.
Each core computes 3 pieces - for each: one bf16 matmul pair (512+256
moving) with W-chunk stationary, PSUM->SBUF cast copies, one 128-partition
DMA. All IO is bf16; out pieces ride both HWDGE rings; everything is sized
so each transfer spreads over all 16 SDMA engines.
"""

import math
import sys
import types

import numpy as np

NCORES = 8
CHEB_K = 64  # chebyshev grid size for the 2D expansion
R = 32       # separation rank (machine precision by ~24)
HROW = 768   # rows per piece (half of padded 1536)
NPIECE = 3   # pieces per core

TRACE = False  # test.py sets True to capture an NTFF profile
LAST_RESULTS = None  # BassKernelResults of the last run (for test.py)

_PROGRAM_CACHE = {}


def _install_ntff_hook():
    try:
        from antenv.axon_hooks import get_axon_ntff_profile_hook  # noqa: F401
        return
    except ImportError:
        pass
    try:
        from trn_agent_boot.trn_boot import _ntff_profile_via_ctypes
        import antenv
        hook = _ntff_profile_via_ctypes('/opt/axon/libaxon_pjrt.so')
        mod = types.ModuleType("antenv.axon_hooks")
        mod._hook = hook
        mod.get_axon_ntff_profile_hook = lambda: mod._hook
        mod.set_axon_ntff_profile_hook = lambda h: setattr(mod, "_hook", h)
        antenv.axon_hooks = mod
        sys.modules["antenv.axon_hooks"] = mod
    except Exception:
        pass


def _build_program():
    """Per-core Bass program (same on all 8 cores): 3 output pieces.

    dram "mw" = [W chunks (3x128) | M row-slices (3x768)] bf16, one entry
    per piece (the host packs each core's W column chunk + M row half).
    Piece k: psum[cols 128, rows 768] = Wc_k^T @ Mv_k, copied (cast bf16)
    to SBUF, DMA'd as one 128-partition transfer.
    """
    import concourse.bacc as bacc
    import concourse.tile as tile
    import concourse.mybir as mybir

    BF16 = mybir.dt.bfloat16
    F32 = mybir.dt.float32

    WOFF = NPIECE * 128               # where the shared M half starts in mw
    MCS = [512, HROW - 512]           # moving chunks per piece (psum <= 512)

    nc = bacc.Bacc("TRN2", target_bir_lowering=False, debug=False)
    mw_h = nc.dram_tensor("mw", [R, WOFF + HROW], BF16, kind="ExternalInput")
    o_h = nc.dram_tensor("o", [128, NPIECE * HROW], BF16,
                         kind="ExternalOutput")

    with tile.TileContext(nc) as tc:
        with (
            tc.tile_pool(name="const", bufs=1) as cpool,
            tc.tile_pool(name="ps5", bufs=3, space="PSUM") as hpool5,
            tc.tile_pool(name="ps2", bufs=3, space="PSUM") as hpool2,
        ):
            # dummy activation: pulls ACT_TABLE_LOAD into the input-DMA wait
            # (otherwise it lands right before the first PSUM copy and stalls
            # the whole copy chain behind its 1.5us table fetch)
            warm = cpool.tile([1, 8], F32)
            nc.vector.memset(warm[:, :], 0.0)
            nc.scalar.activation(warm[:, :], warm[:, :],
                                 mybir.ActivationFunctionType.Square)
            mw = cpool.tile([R, WOFF + HROW], BF16)
            # all 3 pieces share the single 768-row M half -> 74KB input
            nc.sync.dma_start(out=mw[:, :], in_=mw_h[:, :])
            st = cpool.tile([128, NPIECE * HROW], BF16)
            for k in range(NPIECE):
                ma = WOFF
                for mi, mc in enumerate(MCS):
                    mb = ma + mc
                    last = mi == len(MCS) - 1
                    if mc > 256:
                        ps = hpool5.tile([128, 512], F32, tag="p5")
                    else:
                        ps = hpool2.tile([128, 256], F32, tag="p2")
                    nc.tensor.matmul(ps[:, :mc], mw[:, k * 128:(k + 1) * 128],
                                     mw[:, ma:mb], start=True, stop=True)
                    dst0 = k * HROW + ma - WOFF
                    if last:
                        # single DVE copy: only one engine gates the DMA
                        nc.vector.tensor_copy(st[:, dst0:dst0 + mc],
                                              ps[:, :mc])
                    else:
                        # uneven DVE/ACT split: ACT is the slower chain
                        h = 320
                        nc.vector.tensor_copy(st[:, dst0:dst0 + h],
                                              ps[:, :h])
                        nc.scalar.copy(st[:, dst0 + h:dst0 + mc],
                                       ps[:, h:mc])
                    ma = mb
                if k < NPIECE - 1:
                    nc.sync.dma_start(out=o_h[:, k * HROW:(k + 1) * HROW],
                                      in_=st[:, k * HROW:(k + 1) * HROW])
                else:
                    # last piece: two staggered sub-DMAs on both rings so
                    # the final transfer is short and triggers early
                    sp = k * HROW + MCS[0]
                    nc.scalar.dma_start(out=o_h[:, k * HROW:sp],
                                        in_=st[:, k * HROW:sp])
                    nc.sync.dma_start(out=o_h[:, sp:(k + 1) * HROW],
                                      in_=st[:, sp:(k + 1) * HROW])
    nc.compile()
    return nc


def _cheb_factors(d1, d2, c, r):
    """Rank-r separation gg(d1_i - d2_j) ~= Phi[:, i]^T Psi[:, j]."""
    lo = min(d1.min(), d2.min())
    hi = max(d1.max(), d2.max())
    mid = 0.5 * (lo + hi)
    half = 0.5 * (hi - lo) * 1.0000001 + 1e-12

    K = CHEB_K
    m = np.arange(K)
    xg = np.cos(np.pi * (m + 0.5) / K)  # chebyshev roots grid

    def gg_fn(diff):
        e = np.exp(-0.5 * c * diff * diff)
        return (c - diff * diff * c * c) * e

    F = gg_fn(half * (xg[:, None] - xg[None, :]))
    T = np.cos(np.pi * np.outer(m + 0.5, m) / K)  # T[m, p] = T_p(x_m)
    C = (2.0 / K) ** 2 * (T.T @ F @ T)
    C[0, :] *= 0.5
    C[:, 0] *= 0.5
    U, S, Vt = np.linalg.svd(C)
    r = int(min(r, K))
    cu = U[:, :r] * np.sqrt(S[:r])
    cv = Vt[:r].T * np.sqrt(S[:r])
    Phi = np.polynomial.chebyshev.chebval((d1 - mid) / half, cu)  # [r, n1]
    Psi = np.polynomial.chebyshev.chebval((d2 - mid) / half, cv)  # [r, n2]
    return Phi, Psi


def kernel(**inputs):
    global LAST_RESULTS
    import ml_dtypes

    d1 = np.asarray(inputs["d1"], dtype=np.float64).reshape(-1)
    u1 = np.asarray(inputs["u1"], dtype=np.float64)
    d2 = np.asarray(inputs["d2"], dtype=np.float64).reshape(-1)
    u2 = np.asarray(inputs["u2"], dtype=np.float64)
    ls = float(np.asarray(inputs["lengthscale"]).reshape(-1)[0])
    i1 = np.asarray(inputs["i1"]).reshape(-1).astype(np.int64)
    i2 = np.asarray(inputs["i2"]).reshape(-1).astype(np.int64)
    na1 = int(np.asarray(inputs["natoms1"]))
    na2 = int(np.asarray(inputs["natoms2"]))
    c = 1.0 / (ls * ls)

    Phi, Psi = _cheb_factors(d1, d2, c, R)

    # fold u1 + segment-sum over i1 into the row factor M [R, 3*na1]
    NROW = 3 * na1
    NROWP = 2 * HROW
    assert NROW <= NROWP
    Mt = np.zeros((NROWP, R))
    for a in range(3):
        np.add.at(Mt, 3 * i1 + a, (Phi * u1[:, a]).T)
    M = Mt.T  # [R, NROWP]

    # fold u2 + segment-sum over i2 into the column factor W [R, 3*na2]
    NCOL = 3 * na2
    NCHUNK = (NCOL + 127) // 128
    assert NCHUNK * 2 == NCORES * NPIECE
    Wt = np.zeros((NCHUNK * 128, R))
    for b in range(3):
        np.add.at(Wt, 3 * i2 + b, (Psi * u2[:, b]).T)
    W_full = Wt.T  # [R, NCHUNK*128]

    nc = _PROGRAM_CACHE.get("nc")
    if nc is None:
        nc = _build_program()
        _PROGRAM_CACHE["nc"] = nc

    # core c: row half h = c // 4, column chunks q = 3*(c%4) + k -> every
    # core runs 3 [128-col x 768-row] pieces off one shared 768-row M half
    bf16 = ml_dtypes.bfloat16
    WOFF = NPIECE * 128
    in_maps = []
    for cc in range(NCORES):
        h = cc // 4
        mw = np.zeros((R, WOFF + HROW), np.float32)
        for k in range(NPIECE):
            q = NPIECE * (cc % 4) + k
            mw[:, k * 128:(k + 1) * 128] = W_full[:, q * 128:(q + 1) * 128]
        mw[:, WOFF:] = M[:, h * HROW:(h + 1) * HROW]
        in_maps.append({"mw": mw.astype(bf16)})

    from concourse.bass_utils import run_bass_kernel_spmd
    if TRACE:
        _install_ntff_hook()
    try:
        res = run_bass_kernel_spmd(nc, in_maps, core_ids=list(range(NCORES)),
                                   trace=TRACE)
    except Exception:
        # rare transient NRT_EXEC_UNIT_UNRECOVERABLE on a cold device;
        # one retry has always succeeded
        res = run_bass_kernel_spmd(nc, in_maps, core_ids=list(range(NCORES)),
                                   trace=TRACE)
    LAST_RESULTS = res

    out = np.zeros((3 * na1, 3 * na2), np.float32)
    for cc in range(NCORES):
        o = np.asarray(res.results[cc]["o"], dtype=np.float32)
        h = cc // 4
        nrows = min(HROW, NROW - h * HROW)
        for k in range(NPIECE):
            q = NPIECE * (cc % 4) + k
            ncols = min(128, NCOL - q * 128)
            if ncols <= 0 or nrows <= 0:
                continue
            piece = o[:ncols, k * HROW:k * HROW + nrows]
            out[h * HROW:h * HROW + nrows, q * 128:q * 128 + ncols] = piece.T
    return out


# revision 27
# speedup vs baseline: 1.0192x; 1.0192x over previous
"""DSimilarity.gradgrad force-force covariance block on 8 Trainium2 cores.

out[m*3+a, n*3+b] = sum_{i,j} u1[i,a]*u2[j,b]*gg[i,j]*[i1[i]==m]*[i2[j]==n]
with gg[i,j] = (c - c^2 diff^2) * exp(-0.5 c diff^2), diff = d1[i]-d2[j], c=1/l^2.

gg depends only on the scalar difference d1[i]-d2[j], so the 4000x4000 kernel
matrix separates: a 2D Chebyshev expansion of gg on the observed d-range,
truncated by SVD, gives gg ~= sum_k phi_k(d1) psi_k(d2) with rank ~16 at
machine precision (rank 32 used for margin). Folding the scatter matrices in
on the host reduces the whole computation to out = M^T @ W with
M[k, 3m+a] = sum_{i1[i]=m} phi_k(d1_i) u1[i,a]
W[k, 3n+b] = sum_{i2[j]=n} psi_k(d2_j) u2[j,b].

The 1536x1536 (padded) output is tiled into 24 pieces: 12 column chunks of
128 (transposed: out columns on PSUM partitions) x 2 row halves of 768.
Each core computes 3 pieces - for each: one bf16 matmul pair (512+256
moving) with W-chunk stationary, PSUM->SBUF cast copies, one 128-partition
DMA. All IO is bf16; out pieces ride both HWDGE rings; everything is sized
so each transfer spreads over all 16 SDMA engines.
"""

import math
import sys
import types

import numpy as np

NCORES = 8
CHEB_K = 64  # chebyshev grid size for the 2D expansion
R = 32       # separation rank (machine precision by ~24)
HROW = 768   # rows per piece (half of padded 1536)
NPIECE = 3   # pieces per core

TRACE = False  # test.py sets True to capture an NTFF profile
LAST_RESULTS = None  # BassKernelResults of the last run (for test.py)

_PROGRAM_CACHE = {}


def _install_ntff_hook():
    try:
        from antenv.axon_hooks import get_axon_ntff_profile_hook  # noqa: F401
        return
    except ImportError:
        pass
    try:
        from trn_agent_boot.trn_boot import _ntff_profile_via_ctypes
        import antenv
        hook = _ntff_profile_via_ctypes('/opt/axon/libaxon_pjrt.so')
        mod = types.ModuleType("antenv.axon_hooks")
        mod._hook = hook
        mod.get_axon_ntff_profile_hook = lambda: mod._hook
        mod.set_axon_ntff_profile_hook = lambda h: setattr(mod, "_hook", h)
        antenv.axon_hooks = mod
        sys.modules["antenv.axon_hooks"] = mod
    except Exception:
        pass


def _build_program():
    """Per-core Bass program (same on all 8 cores): 3 output pieces.

    dram "mw" = [W chunks (3x128) | M row-slices (3x768)] bf16, one entry
    per piece (the host packs each core's W column chunk + M row half).
    Piece k: psum[cols 128, rows 768] = Wc_k^T @ Mv_k, copied (cast bf16)
    to SBUF, DMA'd as one 128-partition transfer.
    """
    import concourse.bacc as bacc
    import concourse.tile as tile
    import concourse.mybir as mybir

    BF16 = mybir.dt.bfloat16
    F32 = mybir.dt.float32

    WOFF = NPIECE * 128               # where the shared M half starts in mw
    MCS = [512, HROW - 512]           # moving chunks per piece (psum <= 512)

    nc = bacc.Bacc("TRN2", target_bir_lowering=False, debug=False)
    mw_h = nc.dram_tensor("mw", [R, WOFF + HROW], BF16, kind="ExternalInput")
    o_h = nc.dram_tensor("o", [128, NPIECE * HROW], BF16,
                         kind="ExternalOutput")

    with tile.TileContext(nc) as tc:
        with (
            tc.tile_pool(name="const", bufs=1) as cpool,
            tc.tile_pool(name="ps5", bufs=3, space="PSUM") as hpool5,
            tc.tile_pool(name="ps2", bufs=3, space="PSUM") as hpool2,
            tc.tile_pool(name="pw", bufs=2, space="PSUM") as wpool,
        ):
            # dummy activation: pulls ACT_TABLE_LOAD into the input-DMA wait
            # (otherwise it lands right before the first PSUM copy and stalls
            # the whole copy chain behind its 1.5us table fetch)
            warm = cpool.tile([1, 8], F32)
            nc.vector.memset(warm[:, :], 0.0)
            nc.scalar.activation(warm[:, :], warm[:, :],
                                 mybir.ActivationFunctionType.Square)
            mw = cpool.tile([R, WOFF + HROW], BF16)
            # all 3 pieces share the single 768-row M half -> 74KB input
            nc.sync.dma_start(out=mw[:, :], in_=mw_h[:, :])
            st = cpool.tile([128, NPIECE * HROW], BF16)
            # HAM warm-up: keep the PE array busy from engine start through
            # the epilogue so the clock gate opens (1.2 -> 2.4 GHz); nothing
            # reads these psum tiles
            dum = cpool.tile([R, 512], BF16)
            nc.vector.memset(dum[:, :], 0.0)

            def pe_burn(n):
                for _ in range(n):
                    pw = wpool.tile([128, 512], F32, tag="pw")
                    nc.tensor.matmul(pw[:, :], dum[:, :128], dum[:, :],
                                     start=True, stop=True)

            pe_burn(4)
            for k in range(NPIECE):
                ma = WOFF
                for mi, mc in enumerate(MCS):
                    mb = ma + mc
                    last = mi == len(MCS) - 1
                    if mc > 256:
                        ps = hpool5.tile([128, 512], F32, tag="p5")
                    else:
                        ps = hpool2.tile([128, 256], F32, tag="p2")
                    nc.tensor.matmul(ps[:, :mc], mw[:, k * 128:(k + 1) * 128],
                                     mw[:, ma:mb], start=True, stop=True)
                    dst0 = k * HROW + ma - WOFF
                    if last:
                        # single DVE copy: only one engine gates the DMA
                        nc.vector.tensor_copy(st[:, dst0:dst0 + mc],
                                              ps[:, :mc])
                    else:
                        # uneven DVE/ACT split: ACT is the slower chain
                        h = 320
                        nc.vector.tensor_copy(st[:, dst0:dst0 + h],
                                              ps[:, :h])
                        nc.scalar.copy(st[:, dst0 + h:dst0 + mc],
                                       ps[:, h:mc])
                    ma = mb
                if k < NPIECE - 1:
                    nc.sync.dma_start(out=o_h[:, k * HROW:(k + 1) * HROW],
                                      in_=st[:, k * HROW:(k + 1) * HROW])
                else:
                    # last piece: two staggered sub-DMAs on both rings so
                    # the final transfer is short and triggers early
                    sp = k * HROW + MCS[0]
                    nc.scalar.dma_start(out=o_h[:, k * HROW:sp],
                                        in_=st[:, k * HROW:sp])
                    nc.sync.dma_start(out=o_h[:, sp:(k + 1) * HROW],
                                      in_=st[:, sp:(k + 1) * HROW])
            # burn until the final barrier so the zero-phase runs un-throttled
            pe_burn(9)
    nc.compile()
    return nc


def _cheb_factors(d1, d2, c, r):
    """Rank-r separation gg(d1_i - d2_j) ~= Phi[:, i]^T Psi[:, j]."""
    lo = min(d1.min(), d2.min())
    hi = max(d1.max(), d2.max())
    mid = 0.5 * (lo + hi)
    half = 0.5 * (hi - lo) * 1.0000001 + 1e-12

    K = CHEB_K
    m = np.arange(K)
    xg = np.cos(np.pi * (m + 0.5) / K)  # chebyshev roots grid

    def gg_fn(diff):
        e = np.exp(-0.5 * c * diff * diff)
        return (c - diff * diff * c * c) * e

    F = gg_fn(half * (xg[:, None] - xg[None, :]))
    T = np.cos(np.pi * np.outer(m + 0.5, m) / K)  # T[m, p] = T_p(x_m)
    C = (2.0 / K) ** 2 * (T.T @ F @ T)
    C[0, :] *= 0.5
    C[:, 0] *= 0.5
    U, S, Vt = np.linalg.svd(C)
    r = int(min(r, K))
    cu = U[:, :r] * np.sqrt(S[:r])
    cv = Vt[:r].T * np.sqrt(S[:r])
    Phi = np.polynomial.chebyshev.chebval((d1 - mid) / half, cu)  # [r, n1]
    Psi = np.polynomial.chebyshev.chebval((d2 - mid) / half, cv)  # [r, n2]
    return Phi, Psi


def kernel(**inputs):
    global LAST_RESULTS
    import ml_dtypes

    d1 = np.asarray(inputs["d1"], dtype=np.float64).reshape(-1)
    u1 = np.asarray(inputs["u1"], dtype=np.float64)
    d2 = np.asarray(inputs["d2"], dtype=np.float64).reshape(-1)
    u2 = np.asarray(inputs["u2"], dtype=np.float64)
    ls = float(np.asarray(inputs["lengthscale"]).reshape(-1)[0])
    i1 = np.asarray(inputs["i1"]).reshape(-1).astype(np.int64)
    i2 = np.asarray(inputs["i2"]).reshape(-1).astype(np.int64)
    na1 = int(np.asarray(inputs["natoms1"]))
    na2 = int(np.asarray(inputs["natoms2"]))
    c = 1.0 / (ls * ls)

    Phi, Psi = _cheb_factors(d1, d2, c, R)

    # fold u1 + segment-sum over i1 into the row factor M [R, 3*na1]
    NROW = 3 * na1
    NROWP = 2 * HROW
    assert NROW <= NROWP
    Mt = np.zeros((NROWP, R))
    for a in range(3):
        np.add.at(Mt, 3 * i1 + a, (Phi * u1[:, a]).T)
    M = Mt.T  # [R, NROWP]

    # fold u2 + segment-sum over i2 into the column factor W [R, 3*na2]
    NCOL = 3 * na2
    NCHUNK = (NCOL + 127) // 128
    assert NCHUNK * 2 == NCORES * NPIECE
    Wt = np.zeros((NCHUNK * 128, R))
    for b in range(3):
        np.add.at(Wt, 3 * i2 + b, (Psi * u2[:, b]).T)
    W_full = Wt.T  # [R, NCHUNK*128]

    nc = _PROGRAM_CACHE.get("nc")
    if nc is None:
        nc = _build_program()
        _PROGRAM_CACHE["nc"] = nc

    # core c: row half h = c // 4, column chunks q = 3*(c%4) + k -> every
    # core runs 3 [128-col x 768-row] pieces off one shared 768-row M half
    bf16 = ml_dtypes.bfloat16
    WOFF = NPIECE * 128
    in_maps = []
    for cc in range(NCORES):
        h = cc // 4
        mw = np.zeros((R, WOFF + HROW), np.float32)
        for k in range(NPIECE):
            q = NPIECE * (cc % 4) + k
            mw[:, k * 128:(k + 1) * 128] = W_full[:, q * 128:(q + 1) * 128]
        mw[:, WOFF:] = M[:, h * HROW:(h + 1) * HROW]
        in_maps.append({"mw": mw.astype(bf16)})

    from concourse.bass_utils import run_bass_kernel_spmd
    if TRACE:
        _install_ntff_hook()
    try:
        res = run_bass_kernel_spmd(nc, in_maps, core_ids=list(range(NCORES)),
                                   trace=TRACE)
    except Exception:
        # rare transient NRT_EXEC_UNIT_UNRECOVERABLE on a cold device;
        # one retry has always succeeded
        res = run_bass_kernel_spmd(nc, in_maps, core_ids=list(range(NCORES)),
                                   trace=TRACE)
    LAST_RESULTS = res

    out = np.zeros((3 * na1, 3 * na2), np.float32)
    for cc in range(NCORES):
        o = np.asarray(res.results[cc]["o"], dtype=np.float32)
        h = cc // 4
        nrows = min(HROW, NROW - h * HROW)
        for k in range(NPIECE):
            q = NPIECE * (cc % 4) + k
            ncols = min(128, NCOL - q * 128)
            if ncols <= 0 or nrows <= 0:
                continue
            piece = o[:ncols, k * HROW:k * HROW + nrows]
            out[h * HROW:h * HROW + nrows, q * 128:q * 128 + ncols] = piece.T
    return out


# revision 28
# speedup vs baseline: 1.0548x; 1.0349x over previous
"""DSimilarity.gradgrad force-force covariance block on 8 Trainium2 cores.

out[m*3+a, n*3+b] = sum_{i,j} u1[i,a]*u2[j,b]*gg[i,j]*[i1[i]==m]*[i2[j]==n]
with gg[i,j] = (c - c^2 diff^2) * exp(-0.5 c diff^2), diff = d1[i]-d2[j], c=1/l^2.

gg depends only on the scalar difference d1[i]-d2[j], so the 4000x4000 kernel
matrix separates: a 2D Chebyshev expansion of gg on the observed d-range,
truncated by SVD, gives gg ~= sum_k phi_k(d1) psi_k(d2) with rank ~16 at
machine precision (rank 32 used for margin). Folding the scatter matrices in
on the host reduces the whole computation to out = M^T @ W with
M[k, 3m+a] = sum_{i1[i]=m} phi_k(d1_i) u1[i,a]
W[k, 3n+b] = sum_{i2[j]=n} psi_k(d2_j) u2[j,b].

The 1536x1536 (padded) output is tiled into 24 pieces: 12 column chunks of
128 (transposed: out columns on PSUM partitions) x 2 row halves of 768.
Each core computes 3 pieces - for each: one bf16 matmul pair (512+256
moving) with W-chunk stationary, PSUM->SBUF cast copies, one 128-partition
DMA. All IO is bf16; out pieces ride both HWDGE rings; everything is sized
so each transfer spreads over all 16 SDMA engines.
"""

import math
import sys
import types

import numpy as np

NCORES = 8
CHEB_K = 64  # chebyshev grid size for the 2D expansion
R = 32       # separation rank (machine precision by ~24)
HROW = 768   # rows per piece (half of padded 1536)
NPIECE = 3   # pieces per core

TRACE = False  # test.py sets True to capture an NTFF profile
LAST_RESULTS = None  # BassKernelResults of the last run (for test.py)

_PROGRAM_CACHE = {}


def _install_ntff_hook():
    try:
        from antenv.axon_hooks import get_axon_ntff_profile_hook  # noqa: F401
        return
    except ImportError:
        pass
    try:
        from trn_agent_boot.trn_boot import _ntff_profile_via_ctypes
        import antenv
        hook = _ntff_profile_via_ctypes('/opt/axon/libaxon_pjrt.so')
        mod = types.ModuleType("antenv.axon_hooks")
        mod._hook = hook
        mod.get_axon_ntff_profile_hook = lambda: mod._hook
        mod.set_axon_ntff_profile_hook = lambda h: setattr(mod, "_hook", h)
        antenv.axon_hooks = mod
        sys.modules["antenv.axon_hooks"] = mod
    except Exception:
        pass


def _build_program():
    """Per-core Bass program (same on all 8 cores): 3 output pieces.

    dram "mw" = [W chunks (3x128) | M row-slices (3x768)] bf16, one entry
    per piece (the host packs each core's W column chunk + M row half).
    Piece k: psum[cols 128, rows 768] = Wc_k^T @ Mv_k, copied (cast bf16)
    to SBUF, DMA'd as one 128-partition transfer.
    """
    import concourse.bacc as bacc
    import concourse.tile as tile
    import concourse.mybir as mybir

    BF16 = mybir.dt.bfloat16
    F32 = mybir.dt.float32

    WOFF = NPIECE * 128               # where the shared M half starts in mw
    MCS = [512, HROW - 512]           # moving chunks per piece (psum <= 512)

    nc = bacc.Bacc("TRN2", target_bir_lowering=False, debug=False)
    mw_h = nc.dram_tensor("mw", [R, WOFF + HROW], BF16, kind="ExternalInput")
    o_h = nc.dram_tensor("o", [128, NPIECE * HROW], BF16,
                         kind="ExternalOutput")

    with tile.TileContext(nc) as tc:
        with (
            tc.tile_pool(name="const", bufs=1) as cpool,
            tc.tile_pool(name="ps5", bufs=3, space="PSUM") as hpool5,
            tc.tile_pool(name="ps2", bufs=3, space="PSUM") as hpool2,
        ):
            # dummy activation: pulls ACT_TABLE_LOAD into the input-DMA wait
            # (otherwise it lands right before the first PSUM copy and stalls
            # the whole copy chain behind its 1.5us table fetch)
            warm = cpool.tile([1, 8], F32)
            nc.vector.memset(warm[:, :], 0.0)
            nc.scalar.activation(warm[:, :], warm[:, :],
                                 mybir.ActivationFunctionType.Square)
            mw = cpool.tile([R, WOFF + HROW], BF16)
            # all 3 pieces share the single 768-row M half -> 74KB input
            nc.sync.dma_start(out=mw[:, :], in_=mw_h[:, :])
            st = cpool.tile([128, NPIECE * HROW], BF16)
            for k in range(NPIECE):
                ma = WOFF
                for mi, mc in enumerate(MCS):
                    mb = ma + mc
                    last = mi == len(MCS) - 1
                    if mc > 256:
                        ps = hpool5.tile([128, 512], F32, tag="p5")
                    else:
                        ps = hpool2.tile([128, 256], F32, tag="p2")
                    nc.tensor.matmul(ps[:, :mc], mw[:, k * 128:(k + 1) * 128],
                                     mw[:, ma:mb], start=True, stop=True)
                    dst0 = k * HROW + ma - WOFF
                    if last:
                        # single DVE copy: only one engine gates the DMA
                        nc.vector.tensor_copy(st[:, dst0:dst0 + mc],
                                              ps[:, :mc])
                    else:
                        # uneven DVE/ACT split: ACT is the slower chain
                        h = 320
                        nc.vector.tensor_copy(st[:, dst0:dst0 + h],
                                              ps[:, :h])
                        nc.scalar.copy(st[:, dst0 + h:dst0 + mc],
                                       ps[:, h:mc])
                    ma = mb
                if k < NPIECE - 1:
                    nc.sync.dma_start(out=o_h[:, k * HROW:(k + 1) * HROW],
                                      in_=st[:, k * HROW:(k + 1) * HROW])
                else:
                    # last piece: two staggered sub-DMAs on both rings so
                    # the final transfer is short and triggers early
                    sp = k * HROW + MCS[0]
                    nc.scalar.dma_start(out=o_h[:, k * HROW:sp],
                                        in_=st[:, k * HROW:sp])
                    nc.sync.dma_start(out=o_h[:, sp:(k + 1) * HROW],
                                      in_=st[:, sp:(k + 1) * HROW])
    nc.compile()
    return nc


def _cheb_factors(d1, d2, c, r):
    """Rank-r separation gg(d1_i - d2_j) ~= Phi[:, i]^T Psi[:, j]."""
    lo = min(d1.min(), d2.min())
    hi = max(d1.max(), d2.max())
    mid = 0.5 * (lo + hi)
    half = 0.5 * (hi - lo) * 1.0000001 + 1e-12

    K = CHEB_K
    m = np.arange(K)
    xg = np.cos(np.pi * (m + 0.5) / K)  # chebyshev roots grid

    def gg_fn(diff):
        e = np.exp(-0.5 * c * diff * diff)
        return (c - diff * diff * c * c) * e

    F = gg_fn(half * (xg[:, None] - xg[None, :]))
    T = np.cos(np.pi * np.outer(m + 0.5, m) / K)  # T[m, p] = T_p(x_m)
    C = (2.0 / K) ** 2 * (T.T @ F @ T)
    C[0, :] *= 0.5
    C[:, 0] *= 0.5
    U, S, Vt = np.linalg.svd(C)
    r = int(min(r, K))
    cu = U[:, :r] * np.sqrt(S[:r])
    cv = Vt[:r].T * np.sqrt(S[:r])
    Phi = np.polynomial.chebyshev.chebval((d1 - mid) / half, cu)  # [r, n1]
    Psi = np.polynomial.chebyshev.chebval((d2 - mid) / half, cv)  # [r, n2]
    return Phi, Psi


def kernel(**inputs):
    global LAST_RESULTS
    import ml_dtypes

    d1 = np.asarray(inputs["d1"], dtype=np.float64).reshape(-1)
    u1 = np.asarray(inputs["u1"], dtype=np.float64)
    d2 = np.asarray(inputs["d2"], dtype=np.float64).reshape(-1)
    u2 = np.asarray(inputs["u2"], dtype=np.float64)
    ls = float(np.asarray(inputs["lengthscale"]).reshape(-1)[0])
    i1 = np.asarray(inputs["i1"]).reshape(-1).astype(np.int64)
    i2 = np.asarray(inputs["i2"]).reshape(-1).astype(np.int64)
    na1 = int(np.asarray(inputs["natoms1"]))
    na2 = int(np.asarray(inputs["natoms2"]))
    c = 1.0 / (ls * ls)

    Phi, Psi = _cheb_factors(d1, d2, c, R)

    # fold u1 + segment-sum over i1 into the row factor M [R, 3*na1]
    NROW = 3 * na1
    NROWP = 2 * HROW
    assert NROW <= NROWP
    Mt = np.zeros((NROWP, R))
    for a in range(3):
        np.add.at(Mt, 3 * i1 + a, (Phi * u1[:, a]).T)
    M = Mt.T  # [R, NROWP]

    # fold u2 + segment-sum over i2 into the column factor W [R, 3*na2]
    NCOL = 3 * na2
    NCHUNK = (NCOL + 127) // 128
    assert NCHUNK * 2 == NCORES * NPIECE
    Wt = np.zeros((NCHUNK * 128, R))
    for b in range(3):
        np.add.at(Wt, 3 * i2 + b, (Psi * u2[:, b]).T)
    W_full = Wt.T  # [R, NCHUNK*128]

    nc = _PROGRAM_CACHE.get("nc")
    if nc is None:
        nc = _build_program()
        _PROGRAM_CACHE["nc"] = nc

    # core c: row half h = c // 4, column chunks q = 3*(c%4) + k -> every
    # core runs 3 [128-col x 768-row] pieces off one shared 768-row M half
    bf16 = ml_dtypes.bfloat16
    WOFF = NPIECE * 128
    in_maps = []
    for cc in range(NCORES):
        h = cc // 4
        mw = np.zeros((R, WOFF + HROW), np.float32)
        for k in range(NPIECE):
            q = NPIECE * (cc % 4) + k
            mw[:, k * 128:(k + 1) * 128] = W_full[:, q * 128:(q + 1) * 128]
        mw[:, WOFF:] = M[:, h * HROW:(h + 1) * HROW]
        in_maps.append({"mw": mw.astype(bf16)})

    from concourse.bass_utils import run_bass_kernel_spmd
    if TRACE:
        _install_ntff_hook()
    try:
        res = run_bass_kernel_spmd(nc, in_maps, core_ids=list(range(NCORES)),
                                   trace=TRACE)
    except Exception:
        # rare transient NRT_EXEC_UNIT_UNRECOVERABLE on a cold device;
        # one retry has always succeeded
        res = run_bass_kernel_spmd(nc, in_maps, core_ids=list(range(NCORES)),
                                   trace=TRACE)
    LAST_RESULTS = res

    out = np.zeros((3 * na1, 3 * na2), np.float32)
    for cc in range(NCORES):
        o = np.asarray(res.results[cc]["o"], dtype=np.float32)
        h = cc // 4
        nrows = min(HROW, NROW - h * HROW)
        for k in range(NPIECE):
            q = NPIECE * (cc % 4) + k
            ncols = min(128, NCOL - q * 128)
            if ncols <= 0 or nrows <= 0:
                continue
            piece = o[:ncols, k * HROW:k * HROW + nrows]
            out[h * HROW:h * HROW + nrows, q * 128:q * 128 + ncols] = piece.T
    return out
